# revision 71
# baseline (speedup 1.0000x reference)
"""Bahdanau additive attention on 8 Trainium2 NeuronCores.

Problem: B=32, S=1024, H=1024 fp32.
  U_h   = dec @ U_w.T                    [B, H]
  W_s   = enc @ W_w.T                    [B, S, H]
  att   = tanh(U_h[:,None,:] + W_s) @ v  [B, S]
  alpha = softmax(att, axis=1)
  ctx   = einsum('bs,bsh->bh', alpha, enc)

Sharding: data-parallel over B across 8 cores (4 batches per core),
U_w / W_w / v_w replicated.

Per-core kernel (all matmuls in bf16 with fp32 PSUM accumulation):
  - ONE sync-HWDGE DMA queue carries every f32 input in just-in-time
    order: dec, v, enc[0] quarter-loads (1MB each), then W/U row-pair
    loads interleaved with enc[1] quarters, then enc[b+1] during batch
    b and the alpha/ctx output traffic.  Large batched transfers keep
    the stream bandwidth-bound instead of instruction-issue-bound.
  - dummy matmuls on a zero tile warm the PE HAM clock gate (1.2 ->
    2.4 GHz) from ~4us so real matmuls never run at half clock.
  - batch 0's o-tile loop is fused with the weight stream: o-tiles
    2p/2p+1 run as soon as W row-pair p lands; U pairs + U_h ride
    behind on the same arrivals.
  - batches 1-3: per-slot interleave of enc[b+1] quarter loads, DVE
    casts, PE transposes (2 groups per slot), and ctx(b-1).
  - ScalarE tanh with per-partition bias U_hT[o] on PSUM evacuation;
    v-matvec and ctx matmuls use 4-way column-group concurrency
    (N=256 quarters at tile_position (0,32q)).
  - per-batch softmax on a partition-0 staging row; alpha bounced
    through DRAM to transpose it for the context matmul (PE transpose
    for the last batch).
"""

import numpy as np
from contextlib import ExitStack

import concourse.bacc as bacc
import concourse.mybir as mybir
import concourse.tile as tile
from concourse import masks
from concourse.bass_utils import run_bass_kernel_spmd

N_CORES = 8
B = 32
B_L = B // N_CORES  # 4 batches per core
S = 1024
H = 1024
P = 128
NT = 8  # 1024 / 128 tiles
F32 = mybir.dt.float32
BF16 = mybir.dt.bfloat16
AF = mybir.ActivationFunctionType

WARM0 = 24  # initial HAM warm-up matmuls (N=256)
WARM_K = 2  # maintenance warm matmuls per prologue step


def _emit(tc, stop_after="full"):
    nc = tc.nc
    dec = nc.dram_tensor("dec", [B_L, H], F32, kind="ExternalInput").ap()
    enc = nc.dram_tensor("enc", [B_L, S, H], F32, kind="ExternalInput").ap()
    U_w = nc.dram_tensor("U_w", [H, H], F32, kind="ExternalInput").ap()
    W_w = nc.dram_tensor("W_w", [H, H], F32, kind="ExternalInput").ap()
    v_w = nc.dram_tensor("v_w", [H], F32, kind="ExternalInput").ap()
    ctx_out = nc.dram_tensor("ctx", [B_L, H], F32, kind="ExternalOutput").ap()
    alpha_out = nc.dram_tensor("alpha", [B_L, S], F32, kind="ExternalOutput").ap()

    ctx = ExitStack()
    const = ctx.enter_context(tc.tile_pool(name="const", bufs=1))
    natp = ctx.enter_context(tc.tile_pool(name="nat", bufs=2))
    encq = ctx.enter_context(tc.tile_pool(name="encq", bufs=2))
    encNp = ctx.enter_context(tc.tile_pool(name="encN", bufs=3))
    encTp = ctx.enter_context(tc.tile_pool(name="encT", bufs=2))
    tanhp = ctx.enter_context(tc.tile_pool(name="tanh", bufs=2))
    stgp = ctx.enter_context(tc.tile_pool(name="stg", bufs=1))
    mmps = ctx.enter_context(tc.tile_pool(name="mmps", bufs=4, space="PSUM"))
    trps = ctx.enter_context(tc.tile_pool(name="trps", bufs=2, space="PSUM"))
    attps = ctx.enter_context(tc.tile_pool(name="attps", bufs=1, space="PSUM"))

    identb = const.tile([P, P], BF16)
    masks.make_identity(nc, identb[:])

    # --- HAM warm-up: dummy matmuls on a zeroed tile keep the PE clock
    # gate at 2.4 GHz while the prologue DMAs stream in.
    scr = const.tile([P, 256], BF16)
    nc.gpsimd.memset(scr[:], 0.0)

    def warm(n):
        # one PSUM accumulation chain: back-to-back array activity with
        # no intermediate hazard semaphores (parks in the ctx slot)
        wps = attps.tile([P, 256], F32, tag="att1", name="warm_ps")
        for t in range(n):
            nc.tensor.matmul(
                wps[:], scr[:, 0:P], scr[:], start=(t == 0), stop=(t == n - 1)
            )

    warm(WARM0)

    # --- sync queue: dec + v first (tiny, needed early); their DVE
    # casts are also emitted first so the strict-FIFO DVE never parks
    # behind a later enc arrival.
    dec_nat = const.tile([B_L, H], F32)
    nc.sync.dma_start(dec_nat[:], dec[:])
    vT_f = const.tile([P, NT], F32)
    nc.sync.dma_start(vT_f[:], v_w.rearrange("(t p) -> p t", p=P))
    dec_b16 = const.tile([B_L, H], BF16)
    nc.vector.tensor_copy(dec_b16[:], dec_nat[:])
    vT = const.tile([P, NT], BF16)
    nc.vector.tensor_copy(vT[:], vT_f[:])

    # --- persistent SBUF tensors ---
    W_wT = const.tile([P, NT, H], BF16)  # [h_in, h_tile, o]
    U_wT = encTp.tile([P, NT, H], BF16, tag="encT")
    U_hT = const.tile([P, NT, B_L], F32)
    alphaT = const.tile([P, NT, B_L], BF16)
    encT_cur = encTp.tile([P, NT, S], BF16, tag="encT", name="encT_0")
    # natural-layout bf16 enc, ring of 3 batches  [s_in, s_tile, h]
    encN_t = {0: encNp.tile([P, NT, H], BF16, tag="encN", name="encN_0")}

    def emit_enc_half(b, hh):
        """2MB f32 load of enc[b] s-chunks [4hh, 4hh+4): one big DMA
        keeps the stream bandwidth-bound (issue cost and ring waits are
        per-instruction).  The cast splits across VectorE/ScalarE."""
        natt = encq.tile([P, 4, H], F32, tag="encq", name="enc_stg")
        nc.sync.dma_start(
            natt[:],
            enc[b, 4 * hh * P : (4 * hh + 4) * P, :].rearrange(
                "(k p) h -> p k h", p=P
            ),
        )
        nc.vector.tensor_copy(
            encN_t[b][:, 4 * hh : 4 * hh + 4, :], natt[:]
        )

    emit_enc_half(0, 0)
    emit_enc_half(0, 1)

    decT = const.tile([P, NT, B_L], BF16)
    for k in range(NT):
        ps = mmps.tile([P, B_L], BF16, tag="mm", name="dec_ps")
        nc.tensor.transpose(
            ps[:], dec_b16[:, k * P : (k + 1) * P], identb[0:B_L, 0:B_L]
        )
        nc.vector.tensor_copy(decT[:, k, :], ps[:])

    def emit_weight_quad(dst, src_dram, q):
        """Sync-queue 2MB f32 load of rows [4q,4q+4) of a [1024,1024]
        weight, split-engine cast, PE-transpose into dst o-slices."""
        natt = natp.tile([P, 4, H], F32, tag="nat", name="natw")
        nc.sync.dma_start(
            natt[:],
            src_dram[4 * q * P : (4 * q + 4) * P, :].rearrange(
                "(r p) h -> p r h", p=P
            ),
        )
        # cast on DVE only: ScalarE must stay free for the per-slot
        # tanh chain, which paces the whole batch-0 loop
        natb = natp.tile([P, 4, H], BF16, tag="natb", bufs=2)
        nc.vector.tensor_copy(natb[:], natt[:])
        for r in range(4):
            i = 4 * q + r
            ps = trps.tile([P, NT, P], BF16, tag="tr", name="wtr_ps")
            for j in range(NT):
                nc.tensor.transpose(
                    ps[:, j, :], natb[:, r, j * P : (j + 1) * P], identb[:]
                )
            # alternate evacuation engines so the 2-deep PSUM ring
            # never waits a single engine's semaphore round trip
            dst_sl = dst[:, :, i * P : (i + 1) * P]
            if r % 2 == 0:
                nc.vector.tensor_copy(dst_sl, ps[:])
            else:
                nc.scalar.copy(dst_sl, ps[:])

    def emit_transpose_group(encT_b, b, k, alt=False):
        """PE-transpose the 8 bf16 h-tiles of chunk (b, k) into encT_b.
        alt=True sends odd chunks' evacuations to ScalarE (for phases
        where transposes run back-to-back and ScalarE is idle)."""
        ps = trps.tile([P, NT, P], BF16, tag="tr")
        for j in range(NT):
            nc.tensor.transpose(
                ps[:, j, :], encN_t[b][:, k, j * P : (j + 1) * P], identb[:]
            )
        dst_sl = encT_b[:, :, k * P : (k + 1) * P]
        if alt and k % 2 == 1:
            nc.scalar.copy(dst_sl, ps[:])
        else:
            nc.vector.tensor_copy(dst_sl, ps[:])

    def emit_uh(i):
        """U_hT[:, i, :] = sum_j U_wT[:, j, i-slice].T @ decT[:, j, :]."""
        ps = mmps.tile([P, B_L], F32, tag="mm", name="uh_ps")
        for j in range(NT):
            nc.tensor.matmul(
                ps[:],
                U_wT[:, j, i * P : (i + 1) * P],
                decT[:, j, :],
                start=(j == 0),
                stop=(j == NT - 1),
            )
        nc.vector.tensor_copy(U_hT[:, i, :], ps[:])

    def emit_ctx(b):
        """Context matmuls (4-way col groups) + evacuation + output DMA."""
        ps = attps.tile([P, 256], F32, tag="att1", name="ctx_ps")
        for k in range(NT):
            lhsT = alphaT[:, k, b : b + 1]
            for q in range(4):
                nc.tensor.matmul(
                    ps[32 * q : 32 * q + 1, :],
                    lhsT,
                    encN_t[b][:, k, 256 * q : 256 * (q + 1)],
                    start=(k == 0),
                    stop=(k == NT - 1),
                    tile_position=(0, 32 * q),
                )
        ctx_stg = stgp.tile([1, H], F32, tag="ctxstg")
        for q in range(4):
            out_sl = ctx_stg[0:1, 256 * q : 256 * (q + 1)]
            in_sl = ps[32 * q : 32 * q + 1, :]
            if b == B_L - 1:
                # last batch accumulated unnormalized exp: fold the
                # softmax 1/sum into the evacuation
                nc.vector.tensor_scalar_mul(out_sl, in_sl, srec_last[0])
            else:
                nc.vector.tensor_copy(out_sl, in_sl)
        nc.sync.dma_start(ctx_out[b : b + 1, :], ctx_stg[0:1, :])

    if stop_after == "setup":
        dbg = stgp.tile([1, S], F32, tag="ctxstg")
        nc.gpsimd.memset(dbg[:], 0.0)
        for b in range(B_L):
            nc.sync.dma_start(alpha_out[b : b + 1, :], dbg[0:1, :])
            nc.sync.dma_start(ctx_out[b : b + 1, :], dbg[0:1, 0:H])
        ctx.close()
        return

    def emit_matvec(att_ps, i, th):
        for q in range(4):
            nc.tensor.matmul(
                att_ps[32 * q : 32 * q + 1, :],
                vT[:, i : i + 1],
                th[:, 256 * q : 256 * (q + 1)],
                start=(i == 0),
                stop=(i == NT - 1),
                tile_position=(0, 32 * q),
            )

    def emit_softmax_epilogue(b):
        att_ps = att_tiles[b]
        att_stg = stgp.tile([1, S], F32, tag="attstg")
        for q in range(4):
            nc.vector.tensor_copy(
                att_stg[0:1, 256 * q : 256 * (q + 1)],
                att_ps[32 * q : 32 * q + 1, :],
            )
        if stop_after != "phase1":
            # att is tanh-bounded (|att| <= sum|v| ~ 26) so exp cannot
            # overflow fp32: skip the max-subtraction pass entirely.
            exp_stg = stgp.tile([1, S], F32, tag="expstg")
            ssum = stgp.tile([1, 1], F32, tag="ssum")
            nc.scalar.activation(
                exp_stg[:],
                att_stg[:],
                AF.Exp,
                scale=1.0,
                accum_out=ssum[:],
            )
            srec = stgp.tile([1, 1], F32, tag="srec")
            nc.vector.reciprocal(srec[:], ssum[:])
            srec_last[0] = srec[:]
            alpha_stg = stgp.tile([1, S], F32, tag="alphastg")
            nc.vector.tensor_scalar_mul(alpha_stg[:], exp_stg[:], srec[:])
        else:
            alpha_stg = att_stg
        nc.sync.dma_start(alpha_out[b : b + 1, :], alpha_stg[0:1, :])
        if b < B_L - 1:
            # bounce through DRAM to transpose alpha (latency hidden here)
            alphaT_f = stgp.tile([P, NT], F32, tag="alphaTf")
            nc.sync.dma_start(
                alphaT_f[:], alpha_out[b].rearrange("(k p) -> p k", p=P)
            )
            nc.vector.tensor_copy(alphaT[:, :, b], alphaT_f[:])
        else:
            # last batch: transpose the UNNORMALIZED exp on the array
            # (the 1/sum scale is folded into the ctx evacuation), so
            # the reciprocal+normalize runs off the critical path.
            alpha_b16 = stgp.tile([1, S], BF16, tag="alphab16")
            nc.vector.tensor_copy(alpha_b16[:], exp_stg[:])
            # [P, NT, 2] keeps each transpose's PSUM offset 4B-aligned
            ps = mmps.tile([P, NT, 2], BF16, tag="mm", name="alpha_ps")
            for k in range(NT):
                nc.tensor.transpose(
                    ps[:, k, 0:1],
                    alpha_b16[0:1, k * P : (k + 1) * P],
                    identb[0:1, 0:1],
                )
            nc.vector.tensor_copy(alphaT[:, :, b], ps[:, :, 0])

    att_tiles = {}
    srec_last = {}

    # --- enc[0] transposes: all 8 chunks, before b0's loop ---
    for k in range(NT):
        emit_transpose_group(encT_cur, 0, k, alt=True)
        warm(WARM_K)

    # ================= batch loop, o-tile-major =================
    for b in range(B_L):
        encT_next = None
        if b + 1 < B_L:
            encT_next = encTp.tile(
                [P, NT, S], BF16, tag="encT", name=f"encT_{b + 1}"
            )
        att_ps = attps.tile([P, 256], F32, tag="att0", name="att_ps")
        att_tiles[b] = att_ps

        tanh_prev = None
        for i in range(NT):
            if b == 0:
                warm(4 if i < 3 else 1)
                if i % 4 == 0:
                    # fused weight stream: W quad gates o-tiles 4q..4q+3
                    emit_weight_quad(W_wT, W_w, i // 4)
            ps = [
                mmps.tile([P, 512], F32, tag="mm", name=f"mm_ps{c2}")
                for c2 in range(2)
            ]
            for j in range(NT):
                lhsT = W_wT[:, j, i * P : (i + 1) * P]
                for c in range(2):
                    nc.tensor.matmul(
                        ps[c][:],
                        lhsT,
                        encT_cur[:, j, c * 512 : (c + 1) * 512],
                        start=(j == 0),
                        stop=(j == NT - 1),
                    )
            if b == 0:
                if i % 4 == 0:
                    emit_weight_quad(U_wT, U_w, i // 4)
                    for ii in range(i, i + 4):
                        emit_uh(ii)
                elif i in (1, 5):
                    if i == 1:
                        encN_t[1] = encNp.tile(
                            [P, NT, H], BF16, tag="encN", name="encN_1"
                        )
                    emit_enc_half(1, (i - 1) // 4)
            else:
                if i % 4 == 0 and b + 1 < B_L:
                    if i == 0:
                        encN_t[b + 1] = encNp.tile(
                            [P, NT, H], BF16, tag="encN", name=f"encN_{b + 1}"
                        )
                    emit_enc_half(b + 1, i // 4)
                if encT_next is not None:
                    emit_transpose_group(encT_next, b + 1, i, alt=True)
                if i == 5:
                    # slot 5 (not 3): the alpha DRAM bounce behind this
                    # read can lag the softmax; later placement keeps
                    # the in-order PE stream from stalling on it
                    emit_ctx(b - 1)
            if tanh_prev is not None:
                ip, thp = tanh_prev
                emit_matvec(att_ps, ip, thp)
            th = tanhp.tile([P, 1024], BF16, tag="tanh")
            for c in range(2):
                nc.scalar.activation(
                    th[:, c * 512 : (c + 1) * 512],
                    ps[c][:],
                    AF.Tanh,
                    bias=U_hT[:, i, b : b + 1],
                    scale=1.0,
                )
            tanh_prev = (i, th)
        ip, thp = tanh_prev
        emit_matvec(att_ps, ip, thp)
        if b == B_L - 1:
            # keep the PE clock warm through the softmax -> alpha
            # transpose -> ctx(3) tail (PE otherwise idles > 3.4us)
            warm(16)

        if b == 0:
            # enc[1] transposes: quarters arrive too late to interleave
            # into batch 0's slots; do them at the boundary.
            for k in range(NT):
                emit_transpose_group(encT_next, 1, k, alt=True)
        emit_softmax_epilogue(b)
        if encT_next is not None:
            encT_cur = encT_next

    emit_ctx(B_L - 1)
    ctx.close()


_CACHED = None


def _build(stop_after="full"):
    global _CACHED
    if _CACHED is None:
        nc = bacc.Bacc("TRN2", target_bir_lowering=False, debug=False)
        with tile.TileContext(nc) as tc:
            _emit(tc, stop_after=stop_after)
        nc.compile()
        _CACHED = nc
    return _CACHED


def kernel(
    decoder_hidden: np.ndarray,
    encoder_outputs: np.ndarray,
    U_w: np.ndarray,
    W_w: np.ndarray,
    v_w: np.ndarray,
):
    dec = np.ascontiguousarray(np.asarray(decoder_hidden, dtype=np.float32))
    enc = np.ascontiguousarray(np.asarray(encoder_outputs, dtype=np.float32))
    U = np.ascontiguousarray(np.asarray(U_w, dtype=np.float32))
    W = np.ascontiguousarray(np.asarray(W_w, dtype=np.float32))
    v = np.ascontiguousarray(np.asarray(v_w, dtype=np.float32))

    nc = _build()
    in_maps = []
    for c in range(N_CORES):
        sl = slice(c * B_L, (c + 1) * B_L)
        in_maps.append(
            {"dec": dec[sl], "enc": enc[sl], "U_w": U, "W_w": W, "v_w": v}
        )
    res = run_bass_kernel_spmd(nc, in_maps, core_ids=list(range(N_CORES)))
    context = np.concatenate([res.results[c]["ctx"] for c in range(N_CORES)], axis=0)
    alpha = np.concatenate([res.results[c]["alpha"] for c in range(N_CORES)], axis=0)
    return (context.astype(np.float32), alpha.astype(np.float32))


# revision 72
# speedup vs baseline: 1.0194x; 1.0194x over previous
"""Bahdanau additive attention on 8 Trainium2 NeuronCores.

Problem: B=32, S=1024, H=1024 fp32.
  U_h   = dec @ U_w.T                    [B, H]
  W_s   = enc @ W_w.T                    [B, S, H]
  att   = tanh(U_h[:,None,:] + W_s) @ v  [B, S]
  alpha = softmax(att, axis=1)
  ctx   = einsum('bs,bsh->bh', alpha, enc)

Sharding: data-parallel over B across 8 cores (4 batches per core),
U_w / W_w / v_w replicated.

Per-core kernel (all matmuls in bf16 with fp32 PSUM accumulation):
  - ONE sync-HWDGE DMA queue carries every f32 input in just-in-time
    order: dec, v, enc[0] quarter-loads (1MB each), then W/U row-pair
    loads interleaved with enc[1] quarters, then enc[b+1] during batch
    b and the alpha/ctx output traffic.  Large batched transfers keep
    the stream bandwidth-bound instead of instruction-issue-bound.
  - dummy matmuls on a zero tile warm the PE HAM clock gate (1.2 ->
    2.4 GHz) from ~4us so real matmuls never run at half clock.
  - batch 0's o-tile loop is fused with the weight stream: o-tiles
    2p/2p+1 run as soon as W row-pair p lands; U pairs + U_h ride
    behind on the same arrivals.
  - batches 1-3: per-slot interleave of enc[b+1] quarter loads, DVE
    casts, PE transposes (2 groups per slot), and ctx(b-1).
  - ScalarE tanh with per-partition bias U_hT[o] on PSUM evacuation;
    v-matvec and ctx matmuls use 4-way column-group concurrency
    (N=256 quarters at tile_position (0,32q)).
  - per-batch softmax on a partition-0 staging row; alpha bounced
    through DRAM to transpose it for the context matmul (PE transpose
    for the last batch).
"""

import numpy as np
from contextlib import ExitStack

import concourse.bacc as bacc
import concourse.mybir as mybir
import concourse.tile as tile
from concourse import masks
from concourse.bass_utils import run_bass_kernel_spmd

N_CORES = 8
B = 32
B_L = B // N_CORES  # 4 batches per core
S = 1024
H = 1024
P = 128
NT = 8  # 1024 / 128 tiles
F32 = mybir.dt.float32
BF16 = mybir.dt.bfloat16
AF = mybir.ActivationFunctionType

WARM0 = 24  # initial HAM warm-up matmuls (N=256)
WARM_K = 2  # maintenance warm matmuls per prologue step


def _emit(tc, stop_after="full"):
    nc = tc.nc
    dec = nc.dram_tensor("dec", [B_L, H], F32, kind="ExternalInput").ap()
    enc = nc.dram_tensor("enc", [B_L, S, H], F32, kind="ExternalInput").ap()
    U_w = nc.dram_tensor("U_w", [H, H], F32, kind="ExternalInput").ap()
    W_w = nc.dram_tensor("W_w", [H, H], F32, kind="ExternalInput").ap()
    v_w = nc.dram_tensor("v_w", [H], F32, kind="ExternalInput").ap()
    ctx_out = nc.dram_tensor("ctx", [B_L, H], F32, kind="ExternalOutput").ap()
    alpha_out = nc.dram_tensor("alpha", [B_L, S], F32, kind="ExternalOutput").ap()

    ctx = ExitStack()
    const = ctx.enter_context(tc.tile_pool(name="const", bufs=1))
    natp = ctx.enter_context(tc.tile_pool(name="nat", bufs=2))
    encq = ctx.enter_context(tc.tile_pool(name="encq", bufs=2))
    encNp = ctx.enter_context(tc.tile_pool(name="encN", bufs=3))
    encTp = ctx.enter_context(tc.tile_pool(name="encT", bufs=2))
    tanhp = ctx.enter_context(tc.tile_pool(name="tanh", bufs=2))
    stgp = ctx.enter_context(tc.tile_pool(name="stg", bufs=1))
    mmps = ctx.enter_context(tc.tile_pool(name="mmps", bufs=4, space="PSUM"))
    trps = ctx.enter_context(tc.tile_pool(name="trps", bufs=2, space="PSUM"))
    attps = ctx.enter_context(tc.tile_pool(name="attps", bufs=1, space="PSUM"))

    identb = const.tile([P, P], BF16)
    masks.make_identity(nc, identb[:])

    # --- HAM warm-up: dummy matmuls on a zeroed tile keep the PE clock
    # gate at 2.4 GHz while the prologue DMAs stream in.
    scr = const.tile([P, 256], BF16)
    nc.gpsimd.memset(scr[:], 0.0)

    def warm(n):
        # one PSUM accumulation chain: back-to-back array activity with
        # no intermediate hazard semaphores (parks in the ctx slot)
        wps = attps.tile([P, 256], F32, tag="att1", name="warm_ps")
        for t in range(n):
            nc.tensor.matmul(
                wps[:], scr[:, 0:P], scr[:], start=(t == 0), stop=(t == n - 1)
            )

    warm(WARM0)

    # --- sync queue: dec + v first (tiny, needed early); their DVE
    # casts are also emitted first so the strict-FIFO DVE never parks
    # behind a later enc arrival.
    dec_nat = const.tile([B_L, H], F32)
    nc.sync.dma_start(dec_nat[:], dec[:])
    vT_f = const.tile([P, NT], F32)
    nc.sync.dma_start(vT_f[:], v_w.rearrange("(t p) -> p t", p=P))
    dec_b16 = const.tile([B_L, H], BF16)
    nc.vector.tensor_copy(dec_b16[:], dec_nat[:])
    vT = const.tile([P, NT], BF16)
    nc.vector.tensor_copy(vT[:], vT_f[:])

    # --- persistent SBUF tensors ---
    W_wT = const.tile([P, NT, H], BF16)  # [h_in, h_tile, o]
    U_wT = encTp.tile([P, NT, H], BF16, tag="encT")
    U_hT = const.tile([P, NT, B_L], F32)
    alphaT = const.tile([P, NT, B_L], BF16)
    encT_cur = encTp.tile([P, NT, S], BF16, tag="encT", name="encT_0")
    # natural-layout bf16 enc, ring of 3 batches  [s_in, s_tile, h]
    encN_t = {0: encNp.tile([P, NT, H], BF16, tag="encN", name="encN_0")}

    def emit_enc_half(b, hh):
        """2MB f32 load of enc[b] s-chunks [4hh, 4hh+4): one big DMA
        keeps the stream bandwidth-bound (issue cost and ring waits are
        per-instruction).  The cast splits across VectorE/ScalarE."""
        natt = encq.tile([P, 4, H], F32, tag="encq", name="enc_stg")
        nc.sync.dma_start(
            natt[:],
            enc[b, 4 * hh * P : (4 * hh + 4) * P, :].rearrange(
                "(k p) h -> p k h", p=P
            ),
        )
        nc.vector.tensor_copy(
            encN_t[b][:, 4 * hh : 4 * hh + 4, :], natt[:]
        )

    emit_enc_half(0, 0)
    emit_enc_half(0, 1)

    decT = const.tile([P, NT, B_L], BF16)
    for k in range(NT):
        ps = mmps.tile([P, B_L], BF16, tag="mm", name="dec_ps")
        nc.tensor.transpose(
            ps[:], dec_b16[:, k * P : (k + 1) * P], identb[0:B_L, 0:B_L]
        )
        nc.vector.tensor_copy(decT[:, k, :], ps[:])

    def emit_weight_quad(dst, src_dram, q):
        """Sync-queue 2MB f32 load of rows [4q,4q+4) of a [1024,1024]
        weight, split-engine cast, PE-transpose into dst o-slices."""
        natt = natp.tile([P, 4, H], F32, tag="nat", name="natw")
        nc.sync.dma_start(
            natt[:],
            src_dram[4 * q * P : (4 * q + 4) * P, :].rearrange(
                "(r p) h -> p r h", p=P
            ),
        )
        natb = natp.tile([P, 4, H], BF16, tag="natb", bufs=2)
        nc.vector.tensor_copy(natb[:, 0:2, :], natt[:, 0:2, :])
        nc.scalar.copy(natb[:, 2:4, :], natt[:, 2:4, :])
        for r in range(4):
            i = 4 * q + r
            ps = trps.tile([P, NT, P], BF16, tag="tr", name="wtr_ps")
            for j in range(NT):
                nc.tensor.transpose(
                    ps[:, j, :], natb[:, r, j * P : (j + 1) * P], identb[:]
                )
            # alternate evacuation engines so the 2-deep PSUM ring
            # never waits a single engine's semaphore round trip
            dst_sl = dst[:, :, i * P : (i + 1) * P]
            if r % 2 == 0:
                nc.vector.tensor_copy(dst_sl, ps[:])
            else:
                nc.scalar.copy(dst_sl, ps[:])

    def emit_transpose_group(encT_b, b, k, alt=False):
        """PE-transpose the 8 bf16 h-tiles of chunk (b, k) into encT_b.
        alt=True sends odd chunks' evacuations to ScalarE (for phases
        where transposes run back-to-back and ScalarE is idle)."""
        ps = trps.tile([P, NT, P], BF16, tag="tr")
        for j in range(NT):
            nc.tensor.transpose(
                ps[:, j, :], encN_t[b][:, k, j * P : (j + 1) * P], identb[:]
            )
        dst_sl = encT_b[:, :, k * P : (k + 1) * P]
        if alt and k % 2 == 1:
            nc.scalar.copy(dst_sl, ps[:])
        else:
            nc.vector.tensor_copy(dst_sl, ps[:])

    def emit_uh(i):
        """U_hT[:, i, :] = sum_j U_wT[:, j, i-slice].T @ decT[:, j, :]."""
        ps = mmps.tile([P, B_L], F32, tag="mm", name="uh_ps")
        for j in range(NT):
            nc.tensor.matmul(
                ps[:],
                U_wT[:, j, i * P : (i + 1) * P],
                decT[:, j, :],
                start=(j == 0),
                stop=(j == NT - 1),
            )
        nc.vector.tensor_copy(U_hT[:, i, :], ps[:])

    def emit_ctx(b):
        """Context matmuls (4-way col groups) + evacuation + output DMA."""
        ps = attps.tile([P, 256], F32, tag="att1", name="ctx_ps")
        for k in range(NT):
            lhsT = alphaT[:, k, b : b + 1]
            for q in range(4):
                nc.tensor.matmul(
                    ps[32 * q : 32 * q + 1, :],
                    lhsT,
                    encN_t[b][:, k, 256 * q : 256 * (q + 1)],
                    start=(k == 0),
                    stop=(k == NT - 1),
                    tile_position=(0, 32 * q),
                )
        ctx_stg = stgp.tile([1, H], F32, tag="ctxstg")
        for q in range(4):
            out_sl = ctx_stg[0:1, 256 * q : 256 * (q + 1)]
            in_sl = ps[32 * q : 32 * q + 1, :]
            if b == B_L - 1:
                # last batch accumulated unnormalized exp: fold the
                # softmax 1/sum into the evacuation
                nc.vector.tensor_scalar_mul(out_sl, in_sl, srec_last[0])
            else:
                nc.vector.tensor_copy(out_sl, in_sl)
        nc.sync.dma_start(ctx_out[b : b + 1, :], ctx_stg[0:1, :])

    if stop_after == "setup":
        dbg = stgp.tile([1, S], F32, tag="ctxstg")
        nc.gpsimd.memset(dbg[:], 0.0)
        for b in range(B_L):
            nc.sync.dma_start(alpha_out[b : b + 1, :], dbg[0:1, :])
            nc.sync.dma_start(ctx_out[b : b + 1, :], dbg[0:1, 0:H])
        ctx.close()
        return

    def emit_matvec(att_ps, i, th):
        for q in range(4):
            nc.tensor.matmul(
                att_ps[32 * q : 32 * q + 1, :],
                vT[:, i : i + 1],
                th[:, 256 * q : 256 * (q + 1)],
                start=(i == 0),
                stop=(i == NT - 1),
                tile_position=(0, 32 * q),
            )

    def emit_softmax_epilogue(b):
        att_ps = att_tiles[b]
        att_stg = stgp.tile([1, S], F32, tag="attstg")
        for q in range(4):
            nc.vector.tensor_copy(
                att_stg[0:1, 256 * q : 256 * (q + 1)],
                att_ps[32 * q : 32 * q + 1, :],
            )
        if stop_after != "phase1":
            # att is tanh-bounded (|att| <= sum|v| ~ 26) so exp cannot
            # overflow fp32: skip the max-subtraction pass entirely.
            exp_stg = stgp.tile([1, S], F32, tag="expstg")
            ssum = stgp.tile([1, 1], F32, tag="ssum")
            nc.scalar.activation(
                exp_stg[:],
                att_stg[:],
                AF.Exp,
                scale=1.0,
                accum_out=ssum[:],
            )
            srec = stgp.tile([1, 1], F32, tag="srec")
            nc.vector.reciprocal(srec[:], ssum[:])
            srec_last[0] = srec[:]
            alpha_stg = stgp.tile([1, S], F32, tag="alphastg")
            nc.vector.tensor_scalar_mul(alpha_stg[:], exp_stg[:], srec[:])
        else:
            alpha_stg = att_stg
        nc.sync.dma_start(alpha_out[b : b + 1, :], alpha_stg[0:1, :])
        if b < B_L - 1:
            # bounce through DRAM to transpose alpha (latency hidden here)
            alphaT_f = stgp.tile([P, NT], F32, tag="alphaTf")
            nc.sync.dma_start(
                alphaT_f[:], alpha_out[b].rearrange("(k p) -> p k", p=P)
            )
            nc.vector.tensor_copy(alphaT[:, :, b], alphaT_f[:])
        else:
            # last batch: transpose the UNNORMALIZED exp on the array
            # (the 1/sum scale is folded into the ctx evacuation), so
            # the reciprocal+normalize runs off the critical path.
            alpha_b16 = stgp.tile([1, S], BF16, tag="alphab16")
            nc.vector.tensor_copy(alpha_b16[:], exp_stg[:])
            # [P, NT, 2] keeps each transpose's PSUM offset 4B-aligned
            ps = mmps.tile([P, NT, 2], BF16, tag="mm", name="alpha_ps")
            for k in range(NT):
                nc.tensor.transpose(
                    ps[:, k, 0:1],
                    alpha_b16[0:1, k * P : (k + 1) * P],
                    identb[0:1, 0:1],
                )
            nc.vector.tensor_copy(alphaT[:, :, b], ps[:, :, 0])

    att_tiles = {}
    srec_last = {}

    # --- enc[0] transposes: all 8 chunks, before b0's loop ---
    for k in range(NT):
        emit_transpose_group(encT_cur, 0, k, alt=True)
        warm(WARM_K)

    # ================= batch loop, o-tile-major =================
    for b in range(B_L):
        encT_next = None
        if b + 1 < B_L:
            encT_next = encTp.tile(
                [P, NT, S], BF16, tag="encT", name=f"encT_{b + 1}"
            )
        att_ps = attps.tile([P, 256], F32, tag="att0", name="att_ps")
        att_tiles[b] = att_ps

        tanh_prev = None
        for i in range(NT):
            if b == 0:
                warm(1)
                if i % 4 == 0:
                    # fused weight stream: W quad gates o-tiles 4q..4q+3
                    emit_weight_quad(W_wT, W_w, i // 4)
            ps = [
                mmps.tile([P, 512], F32, tag="mm", name=f"mm_ps{c2}")
                for c2 in range(2)
            ]
            for j in range(NT):
                lhsT = W_wT[:, j, i * P : (i + 1) * P]
                for c in range(2):
                    nc.tensor.matmul(
                        ps[c][:],
                        lhsT,
                        encT_cur[:, j, c * 512 : (c + 1) * 512],
                        start=(j == 0),
                        stop=(j == NT - 1),
                    )
            if b == 0:
                if i % 4 == 0:
                    emit_weight_quad(U_wT, U_w, i // 4)
                    for ii in range(i, i + 4):
                        emit_uh(ii)
                elif i in (1, 5):
                    if i == 1:
                        encN_t[1] = encNp.tile(
                            [P, NT, H], BF16, tag="encN", name="encN_1"
                        )
                    emit_enc_half(1, (i - 1) // 4)
            else:
                if i % 4 == 0 and b + 1 < B_L:
                    if i == 0:
                        encN_t[b + 1] = encNp.tile(
                            [P, NT, H], BF16, tag="encN", name=f"encN_{b + 1}"
                        )
                    emit_enc_half(b + 1, i // 4)
                if encT_next is not None:
                    emit_transpose_group(encT_next, b + 1, i, alt=True)
                if i == 3:
                    emit_ctx(b - 1)
            if tanh_prev is not None:
                ip, thp = tanh_prev
                emit_matvec(att_ps, ip, thp)
            th = tanhp.tile([P, 1024], BF16, tag="tanh")
            for c in range(2):
                nc.scalar.activation(
                    th[:, c * 512 : (c + 1) * 512],
                    ps[c][:],
                    AF.Tanh,
                    bias=U_hT[:, i, b : b + 1],
                    scale=1.0,
                )
            tanh_prev = (i, th)
        ip, thp = tanh_prev
        emit_matvec(att_ps, ip, thp)
        if b == B_L - 1:
            # keep the PE clock warm through the softmax -> alpha
            # transpose -> ctx(3) tail (PE otherwise idles > 3.4us)
            warm(16)

        if b == 0:
            # enc[1] transposes: quarters arrive too late to interleave
            # into batch 0's slots; do them at the boundary.
            for k in range(NT):
                emit_transpose_group(encT_next, 1, k, alt=True)
        emit_softmax_epilogue(b)
        if encT_next is not None:
            encT_cur = encT_next

    emit_ctx(B_L - 1)
    ctx.close()


_CACHED = None


def _build(stop_after="full"):
    global _CACHED
    if _CACHED is None:
        nc = bacc.Bacc("TRN2", target_bir_lowering=False, debug=False)
        with tile.TileContext(nc) as tc:
            _emit(tc, stop_after=stop_after)
        nc.compile()
        _CACHED = nc
    return _CACHED


def kernel(
    decoder_hidden: np.ndarray,
    encoder_outputs: np.ndarray,
    U_w: np.ndarray,
    W_w: np.ndarray,
    v_w: np.ndarray,
):
    dec = np.ascontiguousarray(np.asarray(decoder_hidden, dtype=np.float32))
    enc = np.ascontiguousarray(np.asarray(encoder_outputs, dtype=np.float32))
    U = np.ascontiguousarray(np.asarray(U_w, dtype=np.float32))
    W = np.ascontiguousarray(np.asarray(W_w, dtype=np.float32))
    v = np.ascontiguousarray(np.asarray(v_w, dtype=np.float32))

    nc = _build()
    in_maps = []
    for c in range(N_CORES):
        sl = slice(c * B_L, (c + 1) * B_L)
        in_maps.append(
            {"dec": dec[sl], "enc": enc[sl], "U_w": U, "W_w": W, "v_w": v}
        )
    res = run_bass_kernel_spmd(nc, in_maps, core_ids=list(range(N_CORES)))
    context = np.concatenate([res.results[c]["ctx"] for c in range(N_CORES)], axis=0)
    alpha = np.concatenate([res.results[c]["alpha"] for c in range(N_CORES)], axis=0)
    return (context.astype(np.float32), alpha.astype(np.float32))
